# revision 5
# baseline (speedup 1.0000x reference)
"""Trainium2 Bass kernel: Mesh-GNN message passing (nn_Mesh_GNN_41936060678384).

Strategy (8 NeuronCores, node-sharded, zero collectives):
  * Each core owns a 1024-node slice of the adjacency (its column block of
    adj^T) and processes ALL 32768 edges (edge-MLP replicated; it is cheap
    next to the 1 GiB adjacency stream).
  * Host pre-transposes adj -> adjT and casts to fp8e4 (0/1 are exact in
    e4m3) so the PE streams it as the moving operand at 1 byte/element.
    Activations/weights are fp16 (exactness checked: ~5e-4 rel err).
  * Per-core edge order is rotated by c*4096 so core c's first 4096
    processed edges are its designated em_new output slice -- one SPMD
    program, no partition-id branching.
  * Edge endpoint features are gathered on-device with SBUF-source
    dma_gather(transpose=True) from an SBUF-resident fp16 node table,
    yielding feature-major [64, chunk] tiles directly.
  * edge_sum accumulates in two persistent PSUM banks over 256 k-tiles;
    the vm-MLP consumes it via an fp16 hi/lo split (keeps the large
    edge_sum scale at ~fp32 precision through the fp16 matmul).
"""
import sys
for _p in ('/opt/trn_rl_repo',):
    if _p not in sys.path:
        sys.path.insert(0, _p)
import numpy as np
import ml_dtypes

N, E, L, H = 8192, 32768, 64, 64
NCORES = 8
ESH, NSH = E // NCORES, N // NCORES        # 4096 edges, 1024 nodes per core
CHUNK = 512
NCHUNK = E // CHUNK                        # 64
KT = 128
KPC = CHUNK // KT                          # 4 k-tiles per chunk
NKT = E // KT                              # 256
KGRP = 16                                  # k-tiles per adjT DMA group (2 MiB)
CHGRP = KGRP // KPC                        # 4 chunks per adj group
EMG = 8                                    # chunks per emT DMA group

F16 = np.float16
FP8 = ml_dtypes.float8_e4m3


def build_program(loop_k=1):
    """Build + compile the SPMD Bass program. loop_k>1 wraps the whole body
    in a hardware For loop (used only for timing measurements)."""
    import concourse.bacc as bacc
    import concourse.mybir as mybir
    from concourse import tile
    dt = mybir.dt
    AF = mybir.ActivationFunctionType

    nc = bacc.Bacc("TRN2", target_bir_lowering=False, debug=False,
                   enable_asserts=False, num_devices=NCORES)

    adjT = nc.dram_tensor("adjT", [E, NSH], dt.float8e4, kind="ExternalInput")
    emT = nc.dram_tensor("emT", [L, E], dt.float16, kind="ExternalInput")
    vmtab = nc.dram_tensor("vmtab", [128, (N // 128) * 128], dt.float16,
                           kind="ExternalInput")
    vmTs = nc.dram_tensor("vmTs", [L, NSH], dt.float16, kind="ExternalInput")
    sidx = nc.dram_tensor("sidx", [128, E // 16], dt.int16, kind="ExternalInput")
    didx = nc.dram_tensor("didx", [128, E // 16], dt.int16, kind="ExternalInput")
    w1em = nc.dram_tensor("w1em", [L, H], dt.float16, kind="ExternalInput")
    w1s = nc.dram_tensor("w1s", [L, H], dt.float16, kind="ExternalInput")
    w1d = nc.dram_tensor("w1d", [L, H], dt.float16, kind="ExternalInput")
    w2b = nc.dram_tensor("w2b", [H + 1, H], dt.float16, kind="ExternalInput")
    b1e = nc.dram_tensor("b1e", [H, 1], dt.float32, kind="ExternalInput")
    wv1 = nc.dram_tensor("wv1", [2 * L, H], dt.float16, kind="ExternalInput")
    wv1lo = nc.dram_tensor("wv1lo", [L, H], dt.float16, kind="ExternalInput")
    wv2 = nc.dram_tensor("wv2", [H, H], dt.float16, kind="ExternalInput")
    bv1 = nc.dram_tensor("bv1", [H, 1], dt.float32, kind="ExternalInput")
    bv2 = nc.dram_tensor("bv2", [H, 1], dt.float32, kind="ExternalInput")
    em_o = nc.dram_tensor("em_o", [ESH, H], dt.float32, kind="ExternalOutput")
    vm_o = nc.dram_tensor("vm_o", [H, NSH], dt.float32, kind="ExternalOutput")

    with tile.TileContext(nc) as tc:
        with (tc.tile_pool(name="const", bufs=1) as const,
              tc.tile_pool(name="adj", bufs=2) as adjp,
              tc.tile_pool(name="emt", bufs=2) as emp,
              tc.tile_pool(name="gat", bufs=4) as gat,
              tc.tile_pool(name="rt", bufs=2) as rtp,
              tc.tile_pool(name="ze", bufs=6) as zep,
              tc.tile_pool(name="tail", bufs=1) as tailp,
              tc.tile_pool(name="vout", bufs=2) as voutp,
              tc.tile_pool(name="emo", bufs=3) as emop,
              tc.tile_pool(name="yps", bufs=2, space="PSUM") as yps,
              tc.tile_pool(name="zps", bufs=3, space="PSUM") as zps,
              tc.tile_pool(name="esps", bufs=1, space="PSUM") as esps):

            tab = const.tile([128, (N // 128) * 128], dt.float16)
            nc.sync.dma_start(tab[:], vmtab[:])
            six = const.tile([128, E // 16], dt.int16)
            nc.sync.dma_start(six[:], sidx[:])
            dix = const.tile([128, E // 16], dt.int16)
            nc.sync.dma_start(dix[:], didx[:])
            tw1em = const.tile([L, H], dt.float16)
            nc.sync.dma_start(tw1em[:], w1em[:])
            tw1s = const.tile([L, H], dt.float16)
            nc.sync.dma_start(tw1s[:], w1s[:])
            tw1d = const.tile([L, H], dt.float16)
            nc.sync.dma_start(tw1d[:], w1d[:])
            tw2b = const.tile([H + 1, H], dt.float16)
            nc.sync.dma_start(tw2b[:], w2b[:])
            tb1 = const.tile([H, 1], dt.float32)
            nc.sync.dma_start(tb1[:], b1e[:])
            twv1 = const.tile([2 * L, H], dt.float16)
            nc.sync.dma_start(twv1[:], wv1[:])
            twv1lo = const.tile([L, H], dt.float16)
            nc.sync.dma_start(twv1lo[:], wv1lo[:])
            twv2 = const.tile([H, H], dt.float16)
            nc.sync.dma_start(twv2[:], wv2[:])
            tbv1 = const.tile([H, 1], dt.float32)
            nc.sync.dma_start(tbv1[:], bv1[:])
            tbv2 = const.tile([H, 1], dt.float32)
            nc.sync.dma_start(tbv2[:], bv2[:])

            adjT_g = adjT[:].rearrange("(g k p) n -> g p k n", k=KGRP, p=KT)

            def body():
                es0 = esps.tile([H, 512], dt.float32)
                es1 = esps.tile([H, 512], dt.float32)
                adj_t = None
                em_t = None
                for ch in range(NCHUNK):
                    if ch % EMG == 0:
                        em_t = emp.tile([L, EMG * CHUNK], dt.float16)
                        nc.sync.dma_start(
                            em_t[:], emT[:, ch * CHUNK:(ch + EMG) * CHUNK])
                    if ch % CHGRP == 0:
                        adj_t = adjp.tile([128, KGRP * NSH], dt.float8e4)
                        nc.sync.dma_start(
                            adj_t[:].rearrange("p (k n) -> p k n", k=KGRP),
                            adjT_g[ch // CHGRP])
                    sg = gat.tile([128, CHUNK], dt.float16, tag="sg")
                    dg = gat.tile([128, CHUNK], dt.float16, tag="dg")
                    for t_, ix in ((sg, six), (dg, dix)):
                        nc.gpsimd.dma_gather(
                            t_[:].rearrange("p (o n) -> p o n", o=1),
                            tab[:],
                            ix[:, ch * (CHUNK // 16):(ch + 1) * (CHUNK // 16)],
                            num_idxs=CHUNK, num_idxs_reg=CHUNK,
                            elem_size=128, transpose=True,
                            sbuf_tokens_per_rank=128,
                            sbuf_free_dim_per_rank=256)
                    y = yps.tile([H, CHUNK], dt.float32, tag="y")
                    nc.tensor.matmul(
                        y[:], tw1em[:],
                        em_t[:, (ch % EMG) * CHUNK:(ch % EMG + 1) * CHUNK],
                        start=True, stop=False)
                    nc.tensor.matmul(y[:], tw1s[:], sg[0:64, :],
                                     start=False, stop=False)
                    nc.tensor.matmul(y[:], tw1d[:], dg[0:64, :],
                                     start=False, stop=True)
                    rt = rtp.tile([H + 1, CHUNK], dt.float16)
                    nc.scalar.activation(rt[0:H, :], y[:], AF.Relu, bias=tb1[:])
                    nc.vector.memset(rt[H:H + 1, :], 1.0)
                    for j in range(KPC):
                        k = ch * KPC + j
                        zj = zps.tile([KT, H], dt.float32)
                        nc.tensor.matmul(zj[:], rt[:, j * KT:(j + 1) * KT],
                                         tw2b[:], start=True, stop=True)
                        if ch < ESH // CHUNK:
                            eo = emop.tile([KT, H], dt.float32)
                            nc.scalar.copy(eo[:], zj[:])
                            nc.sync.dma_start(em_o[k * KT:(k + 1) * KT, :], eo[:])
                        ze = zep.tile([KT, H], dt.float16)
                        nc.vector.tensor_copy(ze[:], zj[:])
                        kk = k % KGRP
                        nc.tensor.matmul(
                            es0[:], ze[:], adj_t[:, kk * NSH:kk * NSH + 512],
                            start=(k == 0), stop=(k == NKT - 1),
                            skip_group_check=True)
                        nc.tensor.matmul(
                            es1[:], ze[:], adj_t[:, kk * NSH + 512:(kk + 1) * NSH],
                            start=(k == 0), stop=(k == NKT - 1),
                            skip_group_check=True)
                # vm MLP tail on this core's 1024-node slice
                esh = tailp.tile([H, NSH], dt.float16, tag="esh")
                esl = tailp.tile([H, NSH], dt.float16, tag="esl")
                nc.scalar.activation(esh[:, 0:512], es0[:], AF.Identity)
                nc.scalar.activation(esh[:, 512:1024], es1[:], AF.Identity)
                nc.vector.tensor_sub(esl[:, 0:512], es0[:], esh[:, 0:512])
                nc.vector.tensor_sub(esl[:, 512:1024], es1[:], esh[:, 512:1024])
                xv = tailp.tile([2 * L, NSH], dt.float16, tag="xv")
                nc.sync.dma_start(xv[0:L, :], vmTs[:])
                nc.sync.dma_start(xv[L:2 * L, :], esh[:])
                for t in range(NSH // 512):
                    sl = slice(t * 512, (t + 1) * 512)
                    yv = yps.tile([H, 512], dt.float32, tag="y")
                    nc.tensor.matmul(yv[:], twv1[:], xv[:, sl],
                                     start=True, stop=False)
                    nc.tensor.matmul(yv[:], twv1lo[:], esl[:, sl],
                                     start=False, stop=True)
                    rv = rtp.tile([H, 512], dt.float16, tag="rv")
                    nc.scalar.activation(rv[:], yv[:], AF.Relu, bias=tbv1[:])
                    zv = yps.tile([H, 512], dt.float32, tag="y")
                    nc.tensor.matmul(zv[:], twv2[:], rv[:], start=True, stop=True)
                    vo = voutp.tile([H, 512], dt.float32)
                    nc.scalar.activation(vo[:], zv[:], AF.Identity, bias=tbv2[:])
                    nc.sync.dma_start(vm_o[:, sl], vo[:])

            if loop_k == 1:
                body()
            else:
                with tc.For_i(0, loop_k, 1):
                    body()

    nc.compile()
    return nc


def host_prep(inputs):
    """Full (unsharded) numpy inputs -> per-core in_maps."""
    vm = np.asarray(inputs["vm_updated"], dtype=np.float32)
    em = np.asarray(inputs["em_embedded"], dtype=np.float32)
    adj = np.asarray(inputs["adj"], dtype=np.float32)
    src = np.asarray(inputs["edge_src"]).astype(np.int64)
    dst = np.asarray(inputs["edge_dst"]).astype(np.int64)
    W1 = np.asarray(inputs["em_W1"], dtype=np.float32)
    b1 = np.asarray(inputs["em_b1"], dtype=np.float32)
    W2 = np.asarray(inputs["em_W2"], dtype=np.float32)
    b2 = np.asarray(inputs["em_b2"], dtype=np.float32)
    V1 = np.asarray(inputs["vm_W1"], dtype=np.float32)
    c1 = np.asarray(inputs["vm_b1"], dtype=np.float32)
    V2 = np.asarray(inputs["vm_W2"], dtype=np.float32)
    c2 = np.asarray(inputs["vm_b2"], dtype=np.float32)

    adj8 = adj.astype(FP8)
    em16T = np.ascontiguousarray(em.astype(F16).T)           # [64, E]
    vm16 = vm.astype(F16)
    tab = np.zeros((N, 128), dtype=F16)
    tab[:, :64] = vm16
    # SBUF gather-table layout: node i -> partition i%128, rank i//128
    vmtab = np.ascontiguousarray(
        tab.reshape(N // 128, 128, 128).transpose(1, 0, 2).reshape(128, -1))
    vm16T = np.ascontiguousarray(vm16.T)                      # [64, N]

    def wrap_idx(a):
        # index j -> [j%16, j//16], replicated over the 8 Q7 groups
        w = np.ascontiguousarray(a.astype(np.int16).reshape(E // 16, 16).T)
        return np.ascontiguousarray(np.tile(w, (8, 1)))

    shared = {
        "vmtab": vmtab,
        "w1em": np.ascontiguousarray(W1[0:64].astype(F16)),
        "w1s": np.ascontiguousarray(W1[64:128].astype(F16)),
        "w1d": np.ascontiguousarray(W1[128:192].astype(F16)),
        "w2b": np.ascontiguousarray(
            np.vstack([W2, b2[None, :]]).astype(F16)),
        "b1e": np.ascontiguousarray(b1.reshape(H, 1)),
        "wv1": np.ascontiguousarray(V1.astype(F16)),
        "wv1lo": np.ascontiguousarray(V1[64:128].astype(F16)),
        "wv2": np.ascontiguousarray(V2.astype(F16)),
        "bv1": np.ascontiguousarray(c1.reshape(H, 1)),
        "bv2": np.ascontiguousarray(c2.reshape(H, 1)),
    }

    in_maps = []
    for c in range(NCORES):
        r0 = c * ESH
        # adjT slice: rows=edges rotated by r0, cols=this core's node slice
        at = np.ascontiguousarray(adj8[c * NSH:(c + 1) * NSH].T)  # [E, NSH]
        at = np.concatenate([at[r0:], at[:r0]], axis=0)
        emt = np.concatenate([em16T[:, r0:], em16T[:, :r0]], axis=1)
        sr = np.concatenate([src[r0:], src[:r0]])
        dr = np.concatenate([dst[r0:], dst[:r0]])
        m = dict(shared)
        m["adjT"] = np.ascontiguousarray(at)
        m["emT"] = np.ascontiguousarray(emt)
        m["sidx"] = wrap_idx(sr)
        m["didx"] = wrap_idx(dr)
        m["vmTs"] = np.ascontiguousarray(vm16T[:, c * NSH:(c + 1) * NSH])
        in_maps.append(m)
    return in_maps


def unshard(results):
    em_new = np.empty((E, H), dtype=np.float32)
    vm_new = np.empty((N, H), dtype=np.float32)
    for c in range(NCORES):
        em_new[c * ESH:(c + 1) * ESH] = results[c]["em_o"]
        vm_new[c * NSH:(c + 1) * NSH] = results[c]["vm_o"].T
    return vm_new, em_new


_prog_cache = {}


def kernel(**inputs):
    from concourse import bass_utils
    if 1 not in _prog_cache:
        _prog_cache[1] = build_program(loop_k=1)
    nc = _prog_cache[1]
    in_maps = host_prep(inputs)
    res = bass_utils.run_bass_kernel_spmd(nc, in_maps,
                                          core_ids=list(range(NCORES)))
    return unshard(res.results)


# revision 8
# speedup vs baseline: 1.9573x; 1.9573x over previous
"""Trainium2 Bass kernel: Mesh-GNN message passing (nn_Mesh_GNN_41936060678384).

Strategy (8 NeuronCores, node-sharded, zero collectives):
  * Each core owns a 1024-node slice of the adjacency (its column block of
    adj^T) and processes ALL 32768 edges (edge-MLP replicated; it is cheap
    next to the 1 GiB adjacency stream).
  * Host pre-transposes adj -> adjT and casts to fp8e4 (0/1 are exact in
    e4m3) so the PE streams it as the moving operand at 1 byte/element.
    Activations/weights are fp16 (exactness checked: ~5e-4 rel err).
  * Per-core edge order is rotated by c*4096 so core c's first 4096
    processed edges are its designated em_new output slice -- one SPMD
    program, no partition-id branching.
  * Edge endpoint features are gathered on-device with SBUF-source
    dma_gather(transpose=True) from an SBUF-resident fp16 node table,
    yielding feature-major [64, chunk] tiles directly.
  * edge_sum accumulates in two persistent PSUM banks over 256 k-tiles;
    the vm-MLP consumes it via an fp16 hi/lo split (keeps the large
    edge_sum scale at ~fp32 precision through the fp16 matmul).
"""
import sys
for _p in ('/opt/trn_rl_repo',):
    if _p not in sys.path:
        sys.path.insert(0, _p)
import numpy as np
import ml_dtypes

N, E, L, H = 8192, 32768, 64, 64
NCORES = 8
ESH, NSH = E // NCORES, N // NCORES        # 4096 edges, 1024 nodes per core
CHUNK = 512
NCHUNK = E // CHUNK                        # 64
KT = 128
KPC = CHUNK // KT                          # 4 k-tiles per chunk
NKT = E // KT                              # 256
KGRP = 16                                  # k-tiles per adjT DMA group (2 MiB)
CHGRP = KGRP // KPC                        # 4 chunks per adj group
EMG = 8                                    # chunks per emT DMA group

F16 = np.float16
FP8 = ml_dtypes.float8_e4m3


def build_program(loop_k=1, no_gather=False, no_adj_dma=False):
    """Build + compile the SPMD Bass program. loop_k>1 wraps the whole body
    in a hardware For loop (used only for timing measurements)."""
    import concourse.bacc as bacc
    import concourse.mybir as mybir
    from concourse import tile
    dt = mybir.dt
    AF = mybir.ActivationFunctionType

    nc = bacc.Bacc("TRN2", target_bir_lowering=False, debug=False,
                   enable_asserts=False, num_devices=NCORES,
                   num_swdge_queues=4)

    adjT = nc.dram_tensor("adjT", [E, NSH], dt.float8e4, kind="ExternalInput")
    emT = nc.dram_tensor("emT", [L, E], dt.float16, kind="ExternalInput")
    vmtab = nc.dram_tensor("vmtab", [128, (N // 128) * 128], dt.float16,
                           kind="ExternalInput")
    vmTs = nc.dram_tensor("vmTs", [L, NSH], dt.float16, kind="ExternalInput")
    sidx = nc.dram_tensor("sidx", [128, E // 16], dt.int16, kind="ExternalInput")
    didx = nc.dram_tensor("didx", [128, E // 16], dt.int16, kind="ExternalInput")
    w1em = nc.dram_tensor("w1em", [L, H], dt.float16, kind="ExternalInput")
    w1s = nc.dram_tensor("w1s", [L, H], dt.float16, kind="ExternalInput")
    w1d = nc.dram_tensor("w1d", [L, H], dt.float16, kind="ExternalInput")
    w2e = nc.dram_tensor("w2e", [H, H], dt.float16, kind="ExternalInput")
    b2rep = nc.dram_tensor("b2rep", [128, H], dt.float32, kind="ExternalInput")
    b1e = nc.dram_tensor("b1e", [H, 1], dt.float32, kind="ExternalInput")
    wv1 = nc.dram_tensor("wv1", [2 * L, H], dt.float16, kind="ExternalInput")
    wv1lo = nc.dram_tensor("wv1lo", [L, H], dt.float16, kind="ExternalInput")
    wv2 = nc.dram_tensor("wv2", [H, H], dt.float16, kind="ExternalInput")
    bv1 = nc.dram_tensor("bv1", [H, 1], dt.float32, kind="ExternalInput")
    bv2 = nc.dram_tensor("bv2", [H, 1], dt.float32, kind="ExternalInput")
    em_o = nc.dram_tensor("em_o", [ESH, H], dt.float32, kind="ExternalOutput")
    vm_o = nc.dram_tensor("vm_o", [H, NSH], dt.float32, kind="ExternalOutput")

    with tile.TileContext(nc) as tc:
        with (tc.tile_pool(name="const", bufs=1) as const,
              tc.tile_pool(name="adj", bufs=2) as adjp,
              tc.tile_pool(name="emt", bufs=2) as emp,
              tc.tile_pool(name="gat", bufs=4) as gat,
              tc.tile_pool(name="rt", bufs=2) as rtp,
              tc.tile_pool(name="ze", bufs=6) as zep,
              tc.tile_pool(name="tail", bufs=1) as tailp,
              tc.tile_pool(name="vout", bufs=2) as voutp,
              tc.tile_pool(name="emo", bufs=3) as emop,
              tc.tile_pool(name="yps", bufs=2, space="PSUM") as yps,
              tc.tile_pool(name="zps", bufs=3, space="PSUM") as zps,
              tc.tile_pool(name="esps", bufs=1, space="PSUM") as esps):

            tab = const.tile([128, (N // 128) * 128], dt.float16)
            nc.sync.dma_start(tab[:], vmtab[:])
            six = const.tile([128, E // 16], dt.int16)
            nc.sync.dma_start(six[:], sidx[:])
            dix = const.tile([128, E // 16], dt.int16)
            nc.sync.dma_start(dix[:], didx[:])
            tw1em = const.tile([L, H], dt.float16)
            nc.sync.dma_start(tw1em[:], w1em[:])
            tw1s = const.tile([L, H], dt.float16)
            nc.sync.dma_start(tw1s[:], w1s[:])
            tw1d = const.tile([L, H], dt.float16)
            nc.sync.dma_start(tw1d[:], w1d[:])
            tw2e = const.tile([H, H], dt.float16)
            nc.sync.dma_start(tw2e[:], w2e[:])
            tb2r = const.tile([128, H], dt.float32)
            nc.sync.dma_start(tb2r[:], b2rep[:])
            tb1 = const.tile([H, 1], dt.float32)
            nc.sync.dma_start(tb1[:], b1e[:])
            twv1 = const.tile([2 * L, H], dt.float16)
            nc.sync.dma_start(twv1[:], wv1[:])
            twv1lo = const.tile([L, H], dt.float16)
            nc.sync.dma_start(twv1lo[:], wv1lo[:])
            twv2 = const.tile([H, H], dt.float16)
            nc.sync.dma_start(twv2[:], wv2[:])
            tbv1 = const.tile([H, 1], dt.float32)
            nc.sync.dma_start(tbv1[:], bv1[:])
            tbv2 = const.tile([H, 1], dt.float32)
            nc.sync.dma_start(tbv2[:], bv2[:])

            adjT_g = adjT[:].rearrange("(g k p) n -> g p k n", k=KGRP, p=KT)

            def body():
                es0 = esps.tile([H, 512], dt.float32)
                es1 = esps.tile([H, 512], dt.float32)
                adj_t = None
                em_t = None
                for ch in range(NCHUNK):
                    if ch % EMG == 0:
                        em_t = emp.tile([L, EMG * CHUNK], dt.float16)
                        nc.sync.dma_start(
                            em_t[:], emT[:, ch * CHUNK:(ch + EMG) * CHUNK])
                    if ch % CHGRP == 0:
                        if no_adj_dma:
                            if ch == 0:
                                adj_t = adjp.tile([128, KGRP * NSH], dt.float8e4)
                                nc.vector.memset(adj_t[:], 1.0)
                        else:
                            adj_t = adjp.tile([128, KGRP * NSH], dt.float8e4)
                            nc.sync.dma_start(
                                adj_t[:].rearrange("p (k n) -> p k n", k=KGRP),
                                adjT_g[ch // CHGRP])
                    sg = gat.tile([128, CHUNK], dt.float16, tag="sg")
                    dg = gat.tile([128, CHUNK], dt.float16, tag="dg")
                    gpairs = () if no_gather else ((sg, six), (dg, dix))
                    if no_gather:
                        nc.vector.memset(sg[:], 0.25)
                        nc.vector.memset(dg[:], 0.25)
                    for gi, (t_, ix) in enumerate(gpairs):
                        nc.gpsimd.dma_gather(
                            t_[:].rearrange("p (o n) -> p o n", o=1),
                            tab[:],
                            ix[:, ch * (CHUNK // 16):(ch + 1) * (CHUNK // 16)],
                            num_idxs=CHUNK, num_idxs_reg=CHUNK,
                            elem_size=128, transpose=True,
                            sbuf_tokens_per_rank=128,
                            sbuf_free_dim_per_rank=256,
                            queue_num=(2 * ch + gi) % 4)
                    y = yps.tile([H, CHUNK], dt.float32, tag="y")
                    nc.tensor.matmul(
                        y[:], tw1em[:],
                        em_t[:, (ch % EMG) * CHUNK:(ch % EMG + 1) * CHUNK],
                        start=True, stop=False)
                    nc.tensor.matmul(y[:], tw1s[:], sg[0:64, :],
                                     start=False, stop=False)
                    nc.tensor.matmul(y[:], tw1d[:], dg[0:64, :],
                                     start=False, stop=True)
                    rt = rtp.tile([H, CHUNK], dt.float16)
                    nc.scalar.activation(rt[:], y[:], AF.Relu, bias=tb1[:])
                    for j in range(KPC):
                        k = ch * KPC + j
                        zj = zps.tile([KT, H], dt.float32)
                        nc.tensor.matmul(zj[:], rt[:, j * KT:(j + 1) * KT],
                                         tw2e[:], start=True, stop=True)
                        if ch < ESH // CHUNK:
                            eo = emop.tile([KT, H], dt.float32)
                            nc.vector.tensor_add(eo[:], zj[:], tb2r[:, 0:H])
                            nc.sync.dma_start(em_o[k * KT:(k + 1) * KT, :], eo[:])
                        ze = zep.tile([KT, H], dt.float16)
                        nc.vector.tensor_add(ze[:], zj[:], tb2r[:, 0:H])
                        kk = k % KGRP
                        nc.tensor.matmul(
                            es0[:], ze[:], adj_t[:, kk * NSH:kk * NSH + 512],
                            start=(k == 0), stop=(k == NKT - 1),
                            skip_group_check=True)
                        nc.tensor.matmul(
                            es1[:], ze[:], adj_t[:, kk * NSH + 512:(kk + 1) * NSH],
                            start=(k == 0), stop=(k == NKT - 1),
                            skip_group_check=True)
                # vm MLP tail on this core's 1024-node slice
                esh = tailp.tile([H, NSH], dt.float16, tag="esh")
                esl = tailp.tile([H, NSH], dt.float16, tag="esl")
                nc.scalar.activation(esh[:, 0:512], es0[:], AF.Identity)
                nc.scalar.activation(esh[:, 512:1024], es1[:], AF.Identity)
                nc.vector.tensor_sub(esl[:, 0:512], es0[:], esh[:, 0:512])
                nc.vector.tensor_sub(esl[:, 512:1024], es1[:], esh[:, 512:1024])
                xv = tailp.tile([2 * L, NSH], dt.float16, tag="xv")
                nc.sync.dma_start(xv[0:L, :], vmTs[:])
                nc.sync.dma_start(xv[L:2 * L, :], esh[:])
                for t in range(NSH // 512):
                    sl = slice(t * 512, (t + 1) * 512)
                    yv = yps.tile([H, 512], dt.float32, tag="y")
                    nc.tensor.matmul(yv[:], twv1[:], xv[:, sl],
                                     start=True, stop=False)
                    nc.tensor.matmul(yv[:], twv1lo[:], esl[:, sl],
                                     start=False, stop=True)
                    rv = rtp.tile([H, 512], dt.float16, tag="rv")
                    nc.scalar.activation(rv[:], yv[:], AF.Relu, bias=tbv1[:])
                    zv = yps.tile([H, 512], dt.float32, tag="y")
                    nc.tensor.matmul(zv[:], twv2[:], rv[:], start=True, stop=True)
                    vo = voutp.tile([H, 512], dt.float32)
                    nc.scalar.activation(vo[:], zv[:], AF.Identity, bias=tbv2[:])
                    nc.sync.dma_start(vm_o[:, sl], vo[:])

            if loop_k == 1:
                body()
            else:
                with tc.For_i(0, loop_k, 1):
                    body()

    nc.compile()
    return nc


def host_prep(inputs):
    """Full (unsharded) numpy inputs -> per-core in_maps."""
    vm = np.asarray(inputs["vm_updated"], dtype=np.float32)
    em = np.asarray(inputs["em_embedded"], dtype=np.float32)
    adj = np.asarray(inputs["adj"], dtype=np.float32)
    src = np.asarray(inputs["edge_src"]).astype(np.int64)
    dst = np.asarray(inputs["edge_dst"]).astype(np.int64)
    W1 = np.asarray(inputs["em_W1"], dtype=np.float32)
    b1 = np.asarray(inputs["em_b1"], dtype=np.float32)
    W2 = np.asarray(inputs["em_W2"], dtype=np.float32)
    b2 = np.asarray(inputs["em_b2"], dtype=np.float32)
    V1 = np.asarray(inputs["vm_W1"], dtype=np.float32)
    c1 = np.asarray(inputs["vm_b1"], dtype=np.float32)
    V2 = np.asarray(inputs["vm_W2"], dtype=np.float32)
    c2 = np.asarray(inputs["vm_b2"], dtype=np.float32)

    adj8 = adj.astype(FP8)
    em16T = np.ascontiguousarray(em.astype(F16).T)           # [64, E]
    vm16 = vm.astype(F16)
    tab = np.zeros((N, 128), dtype=F16)
    tab[:, :64] = vm16
    # SBUF gather-table layout: node i -> partition i%128, rank i//128
    vmtab = np.ascontiguousarray(
        tab.reshape(N // 128, 128, 128).transpose(1, 0, 2).reshape(128, -1))
    vm16T = np.ascontiguousarray(vm16.T)                      # [64, N]

    def wrap_idx(a):
        # index j -> [j%16, j//16], replicated over the 8 Q7 groups
        w = np.ascontiguousarray(a.astype(np.int16).reshape(E // 16, 16).T)
        return np.ascontiguousarray(np.tile(w, (8, 1)))

    shared = {
        "vmtab": vmtab,
        "w1em": np.ascontiguousarray(W1[0:64].astype(F16)),
        "w1s": np.ascontiguousarray(W1[64:128].astype(F16)),
        "w1d": np.ascontiguousarray(W1[128:192].astype(F16)),
        "w2e": np.ascontiguousarray(W2.astype(F16)),
        "b2rep": np.ascontiguousarray(np.broadcast_to(b2[None, :], (128, H)).astype(np.float32)),
        "b1e": np.ascontiguousarray(b1.reshape(H, 1)),
        "wv1": np.ascontiguousarray(V1.astype(F16)),
        "wv1lo": np.ascontiguousarray(V1[64:128].astype(F16)),
        "wv2": np.ascontiguousarray(V2.astype(F16)),
        "bv1": np.ascontiguousarray(c1.reshape(H, 1)),
        "bv2": np.ascontiguousarray(c2.reshape(H, 1)),
    }

    in_maps = []
    for c in range(NCORES):
        r0 = c * ESH
        # adjT slice: rows=edges rotated by r0, cols=this core's node slice
        at = np.ascontiguousarray(adj8[c * NSH:(c + 1) * NSH].T)  # [E, NSH]
        at = np.concatenate([at[r0:], at[:r0]], axis=0)
        emt = np.concatenate([em16T[:, r0:], em16T[:, :r0]], axis=1)
        sr = np.concatenate([src[r0:], src[:r0]])
        dr = np.concatenate([dst[r0:], dst[:r0]])
        m = dict(shared)
        m["adjT"] = np.ascontiguousarray(at)
        m["emT"] = np.ascontiguousarray(emt)
        m["sidx"] = wrap_idx(sr)
        m["didx"] = wrap_idx(dr)
        m["vmTs"] = np.ascontiguousarray(vm16T[:, c * NSH:(c + 1) * NSH])
        in_maps.append(m)
    return in_maps


def unshard(results):
    em_new = np.empty((E, H), dtype=np.float32)
    vm_new = np.empty((N, H), dtype=np.float32)
    for c in range(NCORES):
        em_new[c * ESH:(c + 1) * ESH] = results[c]["em_o"]
        vm_new[c * NSH:(c + 1) * NSH] = results[c]["vm_o"].T
    return vm_new, em_new


_prog_cache = {}


def kernel(**inputs):
    from concourse import bass_utils
    if 1 not in _prog_cache:
        _prog_cache[1] = build_program(loop_k=1)
    nc = _prog_cache[1]
    in_maps = host_prep(inputs)
    res = bass_utils.run_bass_kernel_spmd(nc, in_maps,
                                          core_ids=list(range(NCORES)))
    return unshard(res.results)
